# revision 41
# baseline (speedup 1.0000x reference)
"""Trainium2 Bass kernel for windowed dual-branch attention (nn_ASSA).

Full inputs in, full outputs out. Shards 2048 windows across 8 NeuronCores
(256 windows / 16384 tokens per core), data-parallel, no collectives.

Per-core pipeline (all matmuls bf16/fp8, fp32 accumulation):
  LN (token-major, exact fp32 stats) -> PE-transpose to feature-major ->
  Q/K feature-major (fp8 out) + V token-major projections ->
  per (512-token quad, head): S = QK^T (fp8 operands, FWL weight loads)
  with relative-position bias and cross-window mask folded into one extra
  accumulate matmul; S computed in two half-PSUM tiles (one per head of the
  pair) -> exp via ACT with fused accum_out denominator; relu via ACT/DVE;
  square on GPSIMD; combine (e*rr + r^2) as ONE fused scalar_tensor_tensor
  per head (rr is a per-partition scalar thanks to the u-split) ->
  PE-transpose A -> AV -> output projection with rank-1 bias matmul ->
  fused residual add -> DMA out.
"""
import sys
sys.path.insert(0, "/opt/trn_rl_repo")
import numpy as np
import ml_dtypes

import concourse.bass as bass
import concourse.tile as tile
from concourse import bacc
from concourse import mybir
from concourse._compat import with_exitstack

F32 = mybir.dt.float32
BF16 = mybir.dt.bfloat16
BF = ml_dtypes.bfloat16

B, H, W, C = 2, 256, 256, 512
WIN, HEADS, HD = 8, 16, 32
N = WIN * WIN
SCALE = HD ** -0.5
EPS = 1e-5
NCORES = 8
NEG = -1e9
FP8 = mybir.dt.float8e4
F8 = ml_dtypes.float8_e4m3
SW = 32.0
SO = 8.0
SQ = 8.0     # extra fp8 scale on q
SK = 8.0     # extra fp8 scale on k
SQK = SQ * SK
SA = 1.0     # scale on combined attention weights (bf16: none)
SV = 1.0     # scale on v (bf16: none)


def _rel_pos_index(w):
    coords = np.stack(np.meshgrid(np.arange(w), np.arange(w), indexing='ij'))
    flat = coords.reshape(2, -1)
    rel = (flat[:, :, None] - flat[:, None, :]).transpose(1, 2, 0)
    rel = rel + (w - 1)
    rel[..., 0] *= 2 * w - 1
    return rel.sum(-1)


@with_exitstack
def _emit(ctx, tc, io, T, stage=99, reps=1):
    nc = tc.nc
    NQ = T // 512           # quads (512 tokens = 4 tbs = 8 windows)

    const = ctx.enter_context(tc.tile_pool(name="const", bufs=1))
    xpool = ctx.enter_context(tc.tile_pool(name="xpool", bufs=13))
    small = ctx.enter_context(tc.tile_pool(name="small", bufs=16))
    xnp = ctx.enter_context(tc.tile_pool(name="xnp", bufs=8))
    xntp = ctx.enter_context(tc.tile_pool(name="xntp", bufs=6))
    qkp = ctx.enter_context(tc.tile_pool(name="qkp", bufs=18))
    vp = ctx.enter_context(tc.tile_pool(name="vp", bufs=10))
    ep = ctx.enter_context(tc.tile_pool(name="ep", bufs=12))
    ap = ctx.enter_context(tc.tile_pool(name="ap", bufs=6))
    atp = ctx.enter_context(tc.tile_pool(name="atp", bufs=7))
    otp = ctx.enter_context(tc.tile_pool(name="otp", bufs=8))
    outp = ctx.enter_context(tc.tile_pool(name="outp", bufs=6))

    # PSUM: 8 banks of 2KB. One shared ring for all 2KB tiles (proj psum,
    # S halves, transposed-A) -- proj tiles are idle during attention, so
    # sharing nearly doubles the attention pipeline depth. AV keeps its own
    # bank (its group tile lives across two pairs).
    ps_main = ctx.enter_context(tc.tile_pool(name="ps_main", bufs=6, space="PSUM"))
    ps_qkv = ps_main
    ps_s = ps_main
    ps_at = ps_main
    ps_av = ctx.enter_context(tc.tile_pool(name="ps_av", bufs=2, space="PSUM"))
    ps_pr = ps_main

    # ---- constants ----
    wq_sb = [const.tile([128, 2, 512], FP8, tag=f"wq{i}", name=f"wq{i}") for i in range(2)]
    wk_sb = [const.tile([128, 2, 512], FP8, tag=f"wk{i}", name=f"wk{i}") for i in range(2)]
    wv_sb = [const.tile([128, 2, 512], FP8, tag=f"wv{i}", name=f"wv{i}") for i in range(2)]
    wp_sb = [const.tile([128, 2, 512], FP8, tag=f"wp{i}", name=f"wp{i}") for i in range(2)]
    for i in range(2):
        nc.sync.dma_start(wq_sb[i][:], io["wq"][i, :, :, :])
        nc.sync.dma_start(wk_sb[i][:], io["wk"][i, :, :, :])
        nc.sync.dma_start(wv_sb[i][:], io["wv"][i, :, :, :])
        nc.sync.dma_start(wp_sb[i][:], io["wp"][i, :, :, :])
    bq_sb = [const.tile([128, 1], F32, tag=f"bq{i}", name=f"bq{i}") for i in range(4)]
    bk_sb = [const.tile([128, 1], F32, tag=f"bk{i}", name=f"bk{i}") for i in range(4)]
    for i in range(4):
        nc.sync.dma_start(bq_sb[i][:], io["bq"][128 * i:128 * (i + 1), :])
        nc.sync.dma_start(bk_sb[i][:], io["bk"][128 * i:128 * (i + 1), :])
    bv_row = const.tile([1, 512], BF16, tag="bv")
    pb_row = const.tile([1, 512], BF16, tag="pb")
    nc.sync.dma_start(bv_row[:], io["bv"][:])
    nc.sync.dma_start(pb_row[:], io["pb"][:])
    rpbf = [const.tile([66, 128], BF16, tag=f"rpbf{h}", name=f"rpbf{h}") for h in range(HEADS)]
    for h in range(HEADS):
        nc.sync.dma_start(rpbf[h][:], io["rpbf"][h, :, :])
    maskr = const.tile([66, 512], BF16, tag="maskr")
    nc.sync.dma_start(maskr[:], io["maskr"][:])
    ident = const.tile([128, 128], BF16, tag="ident")
    nc.sync.dma_start(ident[:], io["ident"][:])
    ident8 = const.tile([128, 128], FP8, tag="ident8")
    nc.sync.dma_start(ident8[:], io["ident8"][:])
    ones_b = const.tile([1, 128], BF16, tag="ones")
    nc.sync.dma_start(ones_b[:], io["ones"][:])

    magic = const.tile([128, 4], mybir.dt.int32, tag="magic")
    nc.vector.memset(magic[:], 0x5f3759df)
    sw1 = float(io["_sw1"])        # sqrt(w1/w0), python float baked in

    import contextlib
    loop_cm = tc.For_i(0, reps, 1) if reps > 1 else contextlib.nullcontext()
    def emit_ln(q):
        """LN for one quad: DMA x, stats, rsqrt, normalize -> (x_t, xn_t)."""
        t0 = q * 512
        x_t = []
        xn_t = []
        mv4 = small.tile([128, 4, 2], F32, tag="mv4", name="mv4")
        for t in range(4):
            xt = xpool.tile([128, 512], F32, tag="x", name="xt")
            nc.sync.dma_start(xt[:], io["x"][t0 + 128 * t: t0 + 128 * (t + 1), :])
            stats = small.tile([128, 6], F32, tag="stats", name="stats")
            nc.vector.bn_stats(out=stats[:], in_=xt[:])
            nc.vector.bn_aggr(out=mv4[:, t, :], in_=stats[:])
            x_t.append(xt)
        # rstd for the whole quad: quake rsqrt + 2 Newton iterations (DVE only)
        ve = small.tile([128, 4], F32, tag="ve", name="ve")
        nc.vector.tensor_scalar(out=ve[:], in0=mv4[:, :, 1], scalar1=EPS,
                                scalar2=None, op0=mybir.AluOpType.add)
        rv = small.tile([128, 4], F32, tag="rv", name="rv")
        nc.vector.reciprocal(out=rv[:], in_=ve[:])
        yy = small.tile([128, 4], F32, tag="yy", name="yy")
        nc.scalar.activation(out=yy[:], in_=rv[:],
                             func=mybir.ActivationFunctionType.Sqrt)
        for t in range(4):
            xn = xnp.tile([128, 512], BF16, tag="xn", name="xn")
            nc.vector.tensor_scalar(out=xn[:], in0=x_t[t][:],
                                    scalar1=mv4[:, t, 0:1], scalar2=yy[:, t:t + 1],
                                    op0=mybir.AluOpType.subtract,
                                    op1=mybir.AluOpType.mult)
            xn_t.append(xn)
        return x_t, xn_t

    DR = mybir.MatmulPerfMode.DoubleRow
    ln_state = {}
    QS = {}        # per-quad tensors
    front = {}     # keyed by global pair index P
    mid = {}

    def emit_qkv(q):
        x_t, xn_t = ln_state.pop(q)
        xnt8 = [xntp.tile([128, 2, 512], FP8, tag="xnt", name="xnt8") for _ in range(2)]
        for c in range(4):
            ps = ps_qkv.tile([128, 512], BF16, tag="ps", name="ps_xnt")
            for t in range(4):
                nc.tensor.transpose(ps[:, 128 * t:128 * (t + 1)],
                                    xn_t[t][:, 128 * c:128 * (c + 1)], ident[:])
            nc.vector.tensor_copy(xnt8[c // 2][:, c % 2, :], ps[:])
        qt = []
        kt = []
        for c in range(4):
            psq = ps_qkv.tile([128, 512], F32, tag="ps", name="ps_q")
            for d in range(2):
                nc.tensor.matmul(psq[:], wq_sb[d][:, :, 128 * c:128 * (c + 1)],
                                 xnt8[d][:], start=(d == 0), stop=(d == 1),
                                 perf_mode=DR)
            qc = qkp.tile([128, 512], FP8, tag="qt")
            nc.scalar.activation(out=qc[:], in_=psq[:],
                                 func=mybir.ActivationFunctionType.Identity,
                                 bias=bq_sb[c][:], scale=SCALE * SQ / SW)
            qt.append(qc)
            psk = ps_qkv.tile([128, 512], F32, tag="ps", name="ps_k")
            for d in range(2):
                nc.tensor.matmul(psk[:], wk_sb[d][:, :, 128 * c:128 * (c + 1)],
                                 xnt8[d][:], start=(d == 0), stop=(d == 1),
                                 perf_mode=DR)
            kc_t = qkp.tile([128, 512], FP8, tag="kt")
            nc.scalar.activation(out=kc_t[:], in_=psk[:],
                                 func=mybir.ActivationFunctionType.Identity,
                                 bias=bk_sb[c][:], scale=SK / SW)
            kt.append(kc_t)
        v_t = []
        for j in range(4):
            psv = ps_qkv.tile([128, 512], F32, tag="ps", name="ps_v")
            for d in range(2):
                nc.tensor.matmul(psv[:], xnt8[d][:, :, 128 * j:128 * (j + 1)],
                                 wv_sb[d][:], start=(d == 0), stop=False,
                                 perf_mode=DR)
            nc.tensor.matmul(psv[:], ones_b[:], bv_row[:], start=False, stop=True)
            vt = vp.tile([128, 512], BF16, tag="v")
            nc.vector.tensor_scalar(out=vt[:], in0=psv[:],
                                    scalar1=1.0 / SW, scalar2=None,
                                    op0=mybir.AluOpType.mult)
            v_t.append(vt)
        ot8 = [otp.tile([128, 2, 4, 128], FP8, tag="ot", name="ot8") for _ in range(2)]
        QS[q] = dict(x_t=x_t, qt=qt, kt=kt, v_t=v_t, ot8=ot8, ps_o=[None] * 4)

    def emit_front(P):
        q, p = divmod(P, 8)
        s = QS[q]
        qt, kt = s["qt"], s["kt"]
        h0 = 2 * p
        ss = [ps_s.tile([128, 4, 128], F32, tag="ps", name="ssu")
              for _ in range(2)]
        # interleave the two heads' QK matmuls across PE row strips
        for u in range(2):
            nc.tensor.matmul(ss[u][:].rearrange("p a b -> p (a b)"),
                             rpbf[h0 + u][:],
                             maskr[:], start=True, stop=False,
                             skip_group_check=True)
        for j in range(4):
            for u in range(2):
                h = h0 + u
                g, i = h // 4, h % 4
                nc.tensor.matmul(ss[u][:, j, :],
                                 qt[g][32 * i:32 * (i + 1), 128 * j:128 * (j + 1)],
                                 kt[g][32 * i:32 * (i + 1), 128 * j:128 * (j + 1)],
                                 start=False, stop=(j == 3),
                                 tile_position=(32 * i, 0), skip_group_check=True)
        e_t = ep.tile([128, 2, 4, 128], BF16, tag="e", name="e_t")
        den = small.tile([128, 2], F32, tag="dn", name="dn")
        r_t = ep.tile([128, 2, 4, 128], BF16, tag="r", name="r_t")
        for u in range(2):
            nc.scalar.activation(out=e_t[:, u, :, :], in_=ss[u][:],
                                 func=mybir.ActivationFunctionType.Exp,
                                 scale=1.0 / SQK,
                                 accum_out=den[:, u:u + 1])
        nc.scalar.activation(out=r_t[:, 0, :, :], in_=ss[0][:],
                             func=mybir.ActivationFunctionType.Relu,
                             scale=sw1 * SA ** 0.5 / SQK)
        nc.vector.tensor_scalar(out=r_t[:, 1, :, :], in0=ss[1][:],
                                scalar1=sw1 * SA ** 0.5 / SQK, scalar2=0.0,
                                op0=mybir.AluOpType.mult,
                                op1=mybir.AluOpType.max)
        r2 = ep.tile([128, 2, 4, 128], BF16, tag="r2", name="r2")
        nc.gpsimd.tensor_mul(r2[:], r_t[:], r_t[:])
        front[P] = (e_t, r2, den)

    def emit_mid_combine(P):
        # combine on DVE only: a2 = e*rr + r2 (fused stt per head)
        e_t, r2, den = front.pop(P)
        rr = small.tile([128, 2], F32, tag="rr", name="rr")
        nc.vector.reciprocal(out=rr[:], in_=den[:])
        a2 = ap.tile([128, 2, 4, 128], BF16, tag="a2", name="a2")
        for u in range(2):
            nc.vector.scalar_tensor_tensor(out=a2[:, u, :, :],
                                           in0=e_t[:, u, :, :],
                                           scalar=rr[:, u:u + 1],
                                           in1=r2[:, u, :, :],
                                           op0=mybir.AluOpType.mult,
                                           op1=mybir.AluOpType.add)
        mid[P] = a2

    def emit_mid_transp(P):
        # PE transposes + PSUM->SBUF copy
        a2 = mid.pop(P)
        pat = ps_at.tile([128, 8, 128], BF16, tag="ps", name="pat")
        for u in range(2):
            for j in range(4):
                nc.tensor.transpose(pat[:, 4 * u + j, :], a2[:, u, j, :],
                                    ident[:])
        at = atp.tile([128, 8, 128], BF16, tag="at", name="at")
        if P % 2 == 0:
            nc.vector.tensor_copy(at[:], pat[:])
        else:
            nc.scalar.copy(at[:], pat[:])
        mid[P] = at

    def emit_back(P):
        q, p = divmod(P, 8)
        s = QS[q]
        v_t, ot8, ps_o = s["v_t"], s["ot8"], s["ps_o"]
        at = mid.pop(P)
        for u in range(2):
            h = 2 * p + u
            g, i = h // 4, h % 4
            if i == 0:
                ps_o[g] = ps_av.tile([128, 4, 128], F32, tag="ps_av", name="ps_av")
            for j in range(4):
                nc.tensor.matmul(ps_o[g][32 * i:32 * (i + 1), j, :],
                                 v_t[j][:, 32 * h:32 * (h + 1)],
                                 at[:, 4 * u + j, :],
                                 start=True, stop=True,
                                 tile_position=(0, 32 * i), skip_group_check=True)
            if i == 3:
                if g % 2 == 0:
                    nc.scalar.activation(out=ot8[g // 2][:, g % 2, :, :],
                                         in_=ps_o[g][:],
                                         func=mybir.ActivationFunctionType.Identity,
                                         scale=SO / (SA * SV))
                else:
                    nc.vector.tensor_scalar(out=ot8[g // 2][:, g % 2, :, :],
                                            in0=ps_o[g][:],
                                            scalar1=SO / (SA * SV), scalar2=None,
                                            op0=mybir.AluOpType.mult)

    def emit_proj(q):
        s = QS.pop(q)
        x_t, ot8 = s["x_t"], s["ot8"]
        t0 = q * 512
        for j in range(4):
            pp = ps_pr.tile([128, 512], F32, tag="ps", name="ps_pr")
            for d in range(2):
                nc.tensor.matmul(pp[:], ot8[d][:, :, j, :], wp_sb[d][:],
                                 start=(d == 0), stop=False, perf_mode=DR)
            nc.tensor.matmul(pp[:], ones_b[:], pb_row[:], start=False, stop=True)
            ot_f = outp.tile([128, 512], F32, tag="out")
            nc.vector.scalar_tensor_tensor(out=ot_f[:], in0=pp[:],
                                           scalar=1.0 / (SW * SO), in1=x_t[j][:],
                                           op0=mybir.AluOpType.mult,
                                           op1=mybir.AluOpType.add)
            nc.sync.dma_start(io["out"][t0 + 128 * j: t0 + 128 * (j + 1), :], ot_f[:])

    with loop_cm:
      for q in range(NQ):
          if q not in ln_state:
              ln_state[q] = emit_ln(q)
          emit_qkv(q)
          for p in range(8):
              P = 8 * q + p
              if P >= 4:
                  emit_back(P - 4)
                  if (P - 4) % 8 == 7:
                      emit_proj((P - 4) // 8)
              if P >= 2:
                  emit_mid_combine(P - 2)
              emit_front(P)
              if p == 3 and q + 1 < NQ:
                  # hoist next quad's LN so its DVE chain completes before
                  # the PE reaches the next quad's xnt transposes
                  ln_state[q + 1] = emit_ln(q + 1)
              if P >= 3:
                  emit_mid_transp(P - 3)
      Pl = 8 * NQ - 1
      emit_mid_combine(Pl - 1)
      emit_mid_transp(Pl - 2)
      emit_mid_combine(Pl)
      emit_mid_transp(Pl - 1)
      emit_mid_transp(Pl)
      for P in (Pl - 3, Pl - 2, Pl - 1, Pl):
          emit_back(P)
      emit_proj(NQ - 1)


def build_nc(T, stage=99, reps=1, sw1=1.0):
    nc = bacc.Bacc()
    io = {
        "x": nc.declare_dram_parameter("x", [T, C], F32, isOutput=False),
        "wq": nc.declare_dram_parameter("wq", [2, 128, 2, C], FP8, isOutput=False),
        "wk": nc.declare_dram_parameter("wk", [2, 128, 2, C], FP8, isOutput=False),
        "wv": nc.declare_dram_parameter("wv", [2, 128, 2, C], FP8, isOutput=False),
        "wp": nc.declare_dram_parameter("wp", [2, 128, 2, C], FP8, isOutput=False),
        "bq": nc.declare_dram_parameter("bq", [C, 1], F32, isOutput=False),
        "bk": nc.declare_dram_parameter("bk", [C, 1], F32, isOutput=False),
        "bv": nc.declare_dram_parameter("bv", [1, C], BF16, isOutput=False),
        "pb": nc.declare_dram_parameter("pb", [1, C], BF16, isOutput=False),
        "rpbf": nc.declare_dram_parameter("rpbf", [HEADS, 66, 128], BF16, isOutput=False),
        "maskr": nc.declare_dram_parameter("maskr", [66, 512], BF16, isOutput=False),
        "ident": nc.declare_dram_parameter("ident", [128, 128], BF16, isOutput=False),
        "ident8": nc.declare_dram_parameter("ident8", [128, 128], FP8, isOutput=False),
        "ones": nc.declare_dram_parameter("ones", [1, 128], BF16, isOutput=False),
        "out": nc.declare_dram_parameter("out", [T, C], F32, isOutput=True),
        "_sw1": sw1,
    }
    with tile.TileContext(nc) as tc:
        _emit(tc, io, T, stage, reps)
    nc.compile()
    return nc


def host_prep(inputs):
    """Shared (replicated) host-side tensors from the raw inputs."""
    wq = inputs["wq"]
    wkv = inputs["wkv"]
    g = inputs["norm1_g"].astype(np.float32)
    b = inputs["norm1_b"].astype(np.float32)
    # fold layernorm affine into the projection weights/biases (exact when
    # g == 1 and b == 0, which is what setup_inputs produces)
    wq_f = (g[:, None] * wq).astype(np.float32)
    wkv_f = (g[:, None] * wkv).astype(np.float32)
    bq_f = (inputs["bq"] + b @ wq).astype(np.float32)
    bkv_f = (inputs["bkv"] + b @ wkv).astype(np.float32)

    wk_f, wv_f = wkv_f[:, :C], wkv_f[:, C:]
    bk_f, bv_f = bkv_f[:C], bkv_f[C:]

    idx = _rel_pos_index(WIN)
    rpb = inputs["bias_table"][idx.reshape(-1)].reshape(N, N, HEADS)
    rpb = rpb.transpose(2, 0, 1).astype(np.float32)       # [h, q, k]

    rpbf = np.zeros((HEADS, 66, 128), np.float32)
    for h in range(HEADS):
        # rows 0..63: row i, col q -> rpb[h][q mod 64, i]
        rpbf[h, :64, :64] = rpb[h].T
        rpbf[h, :64, 64:] = rpb[h].T
    rpbf[:, 64, :64] = 1.0     # q in first window
    rpbf[:, 65, 64:] = 1.0     # q in second window
    rpbf *= SQK                # bias lands on the SQK-scaled logits
    maskr = np.zeros((66, 512), np.float32)
    eye = np.eye(64, dtype=np.float32)
    for j in range(4):
        maskr[:64, 128 * j:128 * j + 64] = eye
        maskr[:64, 128 * j + 64:128 * j + 128] = eye
        maskr[64, 128 * j + 64:128 * j + 128] = NEG   # q-win0 x k-win1
        maskr[65, 128 * j:128 * j + 64] = NEG         # q-win1 x k-win0

    wm = inputs["w_mix"].astype(np.float32)
    e = np.exp(wm - wm.max())
    wmix = e / e.sum()
    w0, w1 = float(wmix[0]), float(wmix[1])

    def pack8(w):
        return np.ascontiguousarray(
            (w * SW).reshape(2, 2, 128, C).transpose(0, 2, 1, 3)).astype(F8)

    shared = {
        "wq": pack8(wq_f), "wk": pack8(wk_f), "wv": pack8(wv_f),
        "wp": pack8(w0 * inputs["proj_w"].astype(np.float32)),
        "bq": (bq_f * SCALE * SQ).reshape(C, 1), "bk": (bk_f * SK).reshape(C, 1),
        "bv": (bv_f * SW).reshape(1, C).astype(BF),
        "pb": (inputs["proj_b"].astype(np.float32) * SW * SO).reshape(1, C).astype(BF),
        "rpbf": rpbf.astype(BF), "maskr": maskr.astype(BF),
        "ident": np.eye(128, dtype=np.float32).astype(BF),
        "ident8": np.eye(128, dtype=np.float32).astype(F8),
        "ones": np.ones((1, 128), np.float32).astype(BF),
    }
    return shared, float(np.sqrt(w1 / w0))


def window_partition(x):
    xw = x.reshape(B, H // WIN, WIN, W // WIN, WIN, C)
    xw = xw.transpose(0, 1, 3, 2, 4, 5).reshape(-1, N, C)
    return np.ascontiguousarray(xw.reshape(-1, C))


def window_reverse(flat):
    o = flat.reshape(B, H // WIN, W // WIN, WIN, WIN, C)
    o = o.transpose(0, 1, 3, 2, 4, 5).reshape(B, H * W, C)
    return np.ascontiguousarray(o)


_NC_CACHE = {}
LAST_RESULTS = None


def kernel(**inputs):
    from concourse.bass_utils import run_bass_kernel_spmd

    inputs = {k: np.asarray(v) for k, v in inputs.items()}
    shared, sw1 = host_prep(inputs)
    xw = window_partition(inputs["x"].astype(np.float32))   # [131072, 512]
    T = xw.shape[0] // NCORES

    key = (T, round(sw1, 6))
    if key not in _NC_CACHE:
        _NC_CACHE[key] = build_nc(T, sw1=sw1)
    nc = _NC_CACHE[key]

    in_maps = []
    for c in range(NCORES):
        m = dict(shared)
        m["x"] = np.ascontiguousarray(xw[c * T:(c + 1) * T])
        in_maps.append(m)
    res = run_bass_kernel_spmd(nc, in_maps, list(range(NCORES)))
    global LAST_RESULTS
    LAST_RESULTS = res
    out = np.concatenate([res.results[c]["out"] for c in range(NCORES)], axis=0)
    return window_reverse(out).astype(np.float32)


# revision 42
# speedup vs baseline: 1.1137x; 1.1137x over previous
"""Trainium2 Bass kernel for windowed dual-branch attention (nn_ASSA).

Full inputs in, full outputs out. Shards 2048 windows across 8 NeuronCores
(256 windows / 16384 tokens per core), data-parallel, no collectives.

Per-core pipeline (all matmuls bf16/fp8, fp32 accumulation):
  LN (token-major, exact fp32 stats) -> PE-transpose to feature-major ->
  Q/K feature-major (fp8 out) + V token-major projections ->
  per (512-token quad, head): S = QK^T (fp8 operands, FWL weight loads)
  with relative-position bias and cross-window mask folded into one extra
  accumulate matmul; S computed in two half-PSUM tiles (one per head of the
  pair) -> exp via ACT with fused accum_out denominator; relu via ACT/DVE;
  square on GPSIMD; combine (e*rr + r^2) as ONE fused scalar_tensor_tensor
  per head (rr is a per-partition scalar thanks to the u-split) ->
  PE-transpose A -> AV -> output projection with rank-1 bias matmul ->
  fused residual add -> DMA out.
"""
import sys
sys.path.insert(0, "/opt/trn_rl_repo")
import numpy as np
import ml_dtypes

import concourse.bass as bass
import concourse.tile as tile
from concourse import bacc
from concourse import mybir
from concourse._compat import with_exitstack

F32 = mybir.dt.float32
BF16 = mybir.dt.bfloat16
BF = ml_dtypes.bfloat16

B, H, W, C = 2, 256, 256, 512
WIN, HEADS, HD = 8, 16, 32
N = WIN * WIN
SCALE = HD ** -0.5
EPS = 1e-5
NCORES = 8
NEG = -1e9
FP8 = mybir.dt.float8e4
F8 = ml_dtypes.float8_e4m3
SW = 32.0
SO = 8.0
SQ = 8.0     # extra fp8 scale on q
SK = 8.0     # extra fp8 scale on k
SQK = SQ * SK
SA = 1.0     # scale on combined attention weights (bf16: none)
SV = 1.0     # scale on v (bf16: none)


def _rel_pos_index(w):
    coords = np.stack(np.meshgrid(np.arange(w), np.arange(w), indexing='ij'))
    flat = coords.reshape(2, -1)
    rel = (flat[:, :, None] - flat[:, None, :]).transpose(1, 2, 0)
    rel = rel + (w - 1)
    rel[..., 0] *= 2 * w - 1
    return rel.sum(-1)


@with_exitstack
def _emit(ctx, tc, io, T, stage=99, reps=1):
    nc = tc.nc
    NQ = T // 512           # quads (512 tokens = 4 tbs = 8 windows)

    const = ctx.enter_context(tc.tile_pool(name="const", bufs=1))
    xpool = ctx.enter_context(tc.tile_pool(name="xpool", bufs=13))
    small = ctx.enter_context(tc.tile_pool(name="small", bufs=16))
    xnp = ctx.enter_context(tc.tile_pool(name="xnp", bufs=8))
    xntp = ctx.enter_context(tc.tile_pool(name="xntp", bufs=6))
    qkp = ctx.enter_context(tc.tile_pool(name="qkp", bufs=18))
    vp = ctx.enter_context(tc.tile_pool(name="vp", bufs=10))
    ep = ctx.enter_context(tc.tile_pool(name="ep", bufs=12))
    ap = ctx.enter_context(tc.tile_pool(name="ap", bufs=6))
    atp = ctx.enter_context(tc.tile_pool(name="atp", bufs=7))
    otp = ctx.enter_context(tc.tile_pool(name="otp", bufs=8))
    outp = ctx.enter_context(tc.tile_pool(name="outp", bufs=6))

    # PSUM: 8 banks of 2KB. One shared ring for all 2KB tiles (proj psum,
    # S halves, transposed-A) -- proj tiles are idle during attention, so
    # sharing nearly doubles the attention pipeline depth. AV keeps its own
    # bank (its group tile lives across two pairs).
    ps_main = ctx.enter_context(tc.tile_pool(name="ps_main", bufs=6, space="PSUM"))
    ps_qkv = ps_main
    ps_s = ps_main
    ps_at = ps_main
    ps_av = ctx.enter_context(tc.tile_pool(name="ps_av", bufs=2, space="PSUM"))
    ps_pr = ps_main

    # ---- constants ----
    wq_sb = [const.tile([128, 2, 512], FP8, tag=f"wq{i}", name=f"wq{i}") for i in range(2)]
    wk_sb = [const.tile([128, 2, 512], FP8, tag=f"wk{i}", name=f"wk{i}") for i in range(2)]
    wv_sb = [const.tile([128, 2, 512], FP8, tag=f"wv{i}", name=f"wv{i}") for i in range(2)]
    wp_sb = [const.tile([128, 2, 512], FP8, tag=f"wp{i}", name=f"wp{i}") for i in range(2)]
    for i in range(2):
        nc.sync.dma_start(wq_sb[i][:], io["wq"][i, :, :, :])
        nc.sync.dma_start(wk_sb[i][:], io["wk"][i, :, :, :])
        nc.sync.dma_start(wv_sb[i][:], io["wv"][i, :, :, :])
        nc.sync.dma_start(wp_sb[i][:], io["wp"][i, :, :, :])
    bq_sb = [const.tile([128, 1], F32, tag=f"bq{i}", name=f"bq{i}") for i in range(4)]
    bk_sb = [const.tile([128, 1], F32, tag=f"bk{i}", name=f"bk{i}") for i in range(4)]
    for i in range(4):
        nc.sync.dma_start(bq_sb[i][:], io["bq"][128 * i:128 * (i + 1), :])
        nc.sync.dma_start(bk_sb[i][:], io["bk"][128 * i:128 * (i + 1), :])
    bv_row = const.tile([1, 512], BF16, tag="bv")
    pb_row = const.tile([1, 512], BF16, tag="pb")
    nc.sync.dma_start(bv_row[:], io["bv"][:])
    nc.sync.dma_start(pb_row[:], io["pb"][:])
    rpbf = [const.tile([66, 128], BF16, tag=f"rpbf{h}", name=f"rpbf{h}") for h in range(HEADS)]
    for h in range(HEADS):
        nc.sync.dma_start(rpbf[h][:], io["rpbf"][h, :, :])
    maskr = const.tile([66, 512], BF16, tag="maskr")
    nc.sync.dma_start(maskr[:], io["maskr"][:])
    ident = const.tile([128, 128], BF16, tag="ident")
    nc.sync.dma_start(ident[:], io["ident"][:])
    ident8 = const.tile([128, 128], FP8, tag="ident8")
    nc.sync.dma_start(ident8[:], io["ident8"][:])
    ones_b = const.tile([1, 128], BF16, tag="ones")
    nc.sync.dma_start(ones_b[:], io["ones"][:])

    magic = const.tile([128, 4], mybir.dt.int32, tag="magic")
    nc.vector.memset(magic[:], 0x5f3759df)
    sw1 = float(io["_sw1"])        # sqrt(w1/w0), python float baked in

    import contextlib
    loop_cm = tc.For_i(0, reps, 1) if reps > 1 else contextlib.nullcontext()
    def emit_ln(q):
        """LN for one quad: DMA x, stats, rsqrt, normalize -> (x_t, xn_t)."""
        t0 = q * 512
        x_t = []
        xn_t = []
        mv4 = small.tile([128, 4, 2], F32, tag="mv4", name="mv4")
        for t in range(4):
            xt = xpool.tile([128, 512], F32, tag="x", name="xt")
            nc.sync.dma_start(xt[:], io["x"][t0 + 128 * t: t0 + 128 * (t + 1), :])
            stats = small.tile([128, 6], F32, tag="stats", name="stats")
            nc.vector.bn_stats(out=stats[:], in_=xt[:])
            nc.vector.bn_aggr(out=mv4[:, t, :], in_=stats[:])
            x_t.append(xt)
        # rstd for the whole quad: quake rsqrt + 2 Newton iterations (DVE only)
        ve = small.tile([128, 4], F32, tag="ve", name="ve")
        nc.vector.tensor_scalar(out=ve[:], in0=mv4[:, :, 1], scalar1=EPS,
                                scalar2=None, op0=mybir.AluOpType.add)
        rv = small.tile([128, 4], F32, tag="rv", name="rv")
        nc.vector.reciprocal(out=rv[:], in_=ve[:])
        yy = small.tile([128, 4], F32, tag="yy", name="yy")
        nc.scalar.activation(out=yy[:], in_=rv[:],
                             func=mybir.ActivationFunctionType.Sqrt)
        for t in range(4):
            xn = xnp.tile([128, 512], BF16, tag="xn", name="xn")
            nc.vector.tensor_scalar(out=xn[:], in0=x_t[t][:],
                                    scalar1=mv4[:, t, 0:1], scalar2=yy[:, t:t + 1],
                                    op0=mybir.AluOpType.subtract,
                                    op1=mybir.AluOpType.mult)
            xn_t.append(xn)
        return x_t, xn_t

    DR = mybir.MatmulPerfMode.DoubleRow
    ln_state = {}
    QS = {}        # per-quad tensors
    front = {}     # keyed by global pair index P
    mid = {}

    def emit_qkv(q):
        x_t, xn_t = ln_state.pop(q)
        xnt8 = [xntp.tile([128, 2, 512], FP8, tag="xnt", name="xnt8") for _ in range(2)]
        for c in range(4):
            ps = ps_qkv.tile([128, 512], BF16, tag="ps", name="ps_xnt")
            for t in range(4):
                nc.tensor.transpose(ps[:, 128 * t:128 * (t + 1)],
                                    xn_t[t][:, 128 * c:128 * (c + 1)], ident[:])
            nc.vector.tensor_copy(xnt8[c // 2][:, c % 2, :], ps[:])
        qt = []
        kt = []
        for c in range(4):
            psq = ps_qkv.tile([128, 512], F32, tag="ps", name="ps_q")
            for d in range(2):
                nc.tensor.matmul(psq[:], wq_sb[d][:, :, 128 * c:128 * (c + 1)],
                                 xnt8[d][:], start=(d == 0), stop=(d == 1),
                                 perf_mode=DR)
            qc = qkp.tile([128, 512], FP8, tag="qt")
            nc.scalar.activation(out=qc[:], in_=psq[:],
                                 func=mybir.ActivationFunctionType.Identity,
                                 bias=bq_sb[c][:], scale=SCALE * SQ / SW)
            qt.append(qc)
            psk = ps_qkv.tile([128, 512], F32, tag="ps", name="ps_k")
            for d in range(2):
                nc.tensor.matmul(psk[:], wk_sb[d][:, :, 128 * c:128 * (c + 1)],
                                 xnt8[d][:], start=(d == 0), stop=(d == 1),
                                 perf_mode=DR)
            kc_t = qkp.tile([128, 512], FP8, tag="kt")
            nc.scalar.activation(out=kc_t[:], in_=psk[:],
                                 func=mybir.ActivationFunctionType.Identity,
                                 bias=bk_sb[c][:], scale=SK / SW)
            kt.append(kc_t)
        v_t = []
        for j in range(4):
            psv = ps_qkv.tile([128, 512], F32, tag="ps", name="ps_v")
            for d in range(2):
                nc.tensor.matmul(psv[:], xnt8[d][:, :, 128 * j:128 * (j + 1)],
                                 wv_sb[d][:], start=(d == 0), stop=False,
                                 perf_mode=DR)
            nc.tensor.matmul(psv[:], ones_b[:], bv_row[:], start=False, stop=True)
            vt = vp.tile([128, 512], BF16, tag="v")
            nc.vector.tensor_scalar(out=vt[:], in0=psv[:],
                                    scalar1=1.0 / SW, scalar2=None,
                                    op0=mybir.AluOpType.mult)
            v_t.append(vt)
        ot8 = [otp.tile([128, 2, 4, 128], FP8, tag="ot", name="ot8") for _ in range(2)]
        QS[q] = dict(x_t=x_t, qt=qt, kt=kt, v_t=v_t, ot8=ot8, ps_o=[None] * 4)

    def emit_front(P):
        q, p = divmod(P, 8)
        s = QS[q]
        qt, kt = s["qt"], s["kt"]
        h0 = 2 * p
        ss = [ps_s.tile([128, 4, 128], F32, tag="ps", name="ssu")
              for _ in range(2)]
        # interleave the two heads' QK matmuls across PE row strips
        for j in range(4):
            for u in range(2):
                h = h0 + u
                g, i = h // 4, h % 4
                nc.tensor.matmul(ss[u][:, j, :],
                                 qt[g][32 * i:32 * (i + 1), 128 * j:128 * (j + 1)],
                                 kt[g][32 * i:32 * (i + 1), 128 * j:128 * (j + 1)],
                                 start=(j == 0), stop=False,
                                 tile_position=(32 * i, 0), skip_group_check=True)
        for u in range(2):
            nc.tensor.matmul(ss[u][:].rearrange("p a b -> p (a b)"),
                             rpbf[h0 + u][:],
                             maskr[:], start=False, stop=True,
                             skip_group_check=True)
        e_t = ep.tile([128, 2, 4, 128], BF16, tag="e", name="e_t")
        den = small.tile([128, 2], F32, tag="dn", name="dn")
        r_t = ep.tile([128, 2, 4, 128], BF16, tag="r", name="r_t")
        for u in range(2):
            nc.scalar.activation(out=e_t[:, u, :, :], in_=ss[u][:],
                                 func=mybir.ActivationFunctionType.Exp,
                                 scale=1.0 / SQK,
                                 accum_out=den[:, u:u + 1])
        nc.scalar.activation(out=r_t[:, 0, :, :], in_=ss[0][:],
                             func=mybir.ActivationFunctionType.Relu,
                             scale=sw1 * SA ** 0.5 / SQK)
        nc.vector.tensor_scalar(out=r_t[:, 1, :, :], in0=ss[1][:],
                                scalar1=sw1 * SA ** 0.5 / SQK, scalar2=0.0,
                                op0=mybir.AluOpType.mult,
                                op1=mybir.AluOpType.max)
        r2 = ep.tile([128, 2, 4, 128], BF16, tag="r2", name="r2")
        nc.gpsimd.tensor_mul(r2[:], r_t[:], r_t[:])
        front[P] = (e_t, r2, den)

    def emit_mid_combine(P):
        # combine on DVE only: a2 = e*rr + r2 (fused stt per head)
        e_t, r2, den = front.pop(P)
        rr = small.tile([128, 2], F32, tag="rr", name="rr")
        nc.vector.reciprocal(out=rr[:], in_=den[:])
        a2 = ap.tile([128, 2, 4, 128], BF16, tag="a2", name="a2")
        for u in range(2):
            nc.vector.scalar_tensor_tensor(out=a2[:, u, :, :],
                                           in0=e_t[:, u, :, :],
                                           scalar=rr[:, u:u + 1],
                                           in1=r2[:, u, :, :],
                                           op0=mybir.AluOpType.mult,
                                           op1=mybir.AluOpType.add)
        mid[P] = a2

    def emit_mid_transp(P):
        # PE transposes + PSUM->SBUF copy
        a2 = mid.pop(P)
        pat = ps_at.tile([128, 8, 128], BF16, tag="ps", name="pat")
        for u in range(2):
            for j in range(4):
                nc.tensor.transpose(pat[:, 4 * u + j, :], a2[:, u, j, :],
                                    ident[:])
        at = atp.tile([128, 8, 128], BF16, tag="at", name="at")
        if P % 2 == 0:
            nc.vector.tensor_copy(at[:], pat[:])
        else:
            nc.scalar.copy(at[:], pat[:])
        mid[P] = at

    def emit_back(P):
        q, p = divmod(P, 8)
        s = QS[q]
        v_t, ot8, ps_o = s["v_t"], s["ot8"], s["ps_o"]
        at = mid.pop(P)
        for u in range(2):
            h = 2 * p + u
            g, i = h // 4, h % 4
            if i == 0:
                ps_o[g] = ps_av.tile([128, 4, 128], F32, tag="ps_av", name="ps_av")
            for j in range(4):
                nc.tensor.matmul(ps_o[g][32 * i:32 * (i + 1), j, :],
                                 v_t[j][:, 32 * h:32 * (h + 1)],
                                 at[:, 4 * u + j, :],
                                 start=True, stop=True,
                                 tile_position=(0, 32 * i), skip_group_check=True)
            if i == 3:
                if g % 2 == 0:
                    nc.scalar.activation(out=ot8[g // 2][:, g % 2, :, :],
                                         in_=ps_o[g][:],
                                         func=mybir.ActivationFunctionType.Identity,
                                         scale=SO / (SA * SV))
                else:
                    nc.vector.tensor_scalar(out=ot8[g // 2][:, g % 2, :, :],
                                            in0=ps_o[g][:],
                                            scalar1=SO / (SA * SV), scalar2=None,
                                            op0=mybir.AluOpType.mult)

    def emit_proj(q):
        s = QS.pop(q)
        x_t, ot8 = s["x_t"], s["ot8"]
        t0 = q * 512
        for j in range(4):
            pp = ps_pr.tile([128, 512], F32, tag="ps", name="ps_pr")
            for d in range(2):
                nc.tensor.matmul(pp[:], ot8[d][:, :, j, :], wp_sb[d][:],
                                 start=(d == 0), stop=False, perf_mode=DR)
            nc.tensor.matmul(pp[:], ones_b[:], pb_row[:], start=False, stop=True)
            ot_f = outp.tile([128, 512], F32, tag="out")
            nc.vector.scalar_tensor_tensor(out=ot_f[:], in0=pp[:],
                                           scalar=1.0 / (SW * SO), in1=x_t[j][:],
                                           op0=mybir.AluOpType.mult,
                                           op1=mybir.AluOpType.add)
            nc.sync.dma_start(io["out"][t0 + 128 * j: t0 + 128 * (j + 1), :], ot_f[:])

    with loop_cm:
      for q in range(NQ):
          if q not in ln_state:
              ln_state[q] = emit_ln(q)
          emit_qkv(q)
          for p in range(8):
              P = 8 * q + p
              if P >= 4:
                  emit_back(P - 4)
                  if (P - 4) % 8 == 7:
                      emit_proj((P - 4) // 8)
              if P >= 3:
                  emit_mid_combine(P - 3)
              emit_front(P)
              if p == 3 and q + 1 < NQ:
                  # hoist next quad's LN so its DVE chain completes before
                  # the PE reaches the next quad's xnt transposes
                  ln_state[q + 1] = emit_ln(q + 1)
              if P >= 3:
                  emit_mid_transp(P - 3)
      Pl = 8 * NQ - 1
      for P in (Pl - 2, Pl - 1, Pl):
          emit_mid_combine(P)
          emit_mid_transp(P)
      for P in (Pl - 3, Pl - 2, Pl - 1, Pl):
          emit_back(P)
      emit_proj(NQ - 1)


def build_nc(T, stage=99, reps=1, sw1=1.0):
    nc = bacc.Bacc()
    io = {
        "x": nc.declare_dram_parameter("x", [T, C], F32, isOutput=False),
        "wq": nc.declare_dram_parameter("wq", [2, 128, 2, C], FP8, isOutput=False),
        "wk": nc.declare_dram_parameter("wk", [2, 128, 2, C], FP8, isOutput=False),
        "wv": nc.declare_dram_parameter("wv", [2, 128, 2, C], FP8, isOutput=False),
        "wp": nc.declare_dram_parameter("wp", [2, 128, 2, C], FP8, isOutput=False),
        "bq": nc.declare_dram_parameter("bq", [C, 1], F32, isOutput=False),
        "bk": nc.declare_dram_parameter("bk", [C, 1], F32, isOutput=False),
        "bv": nc.declare_dram_parameter("bv", [1, C], BF16, isOutput=False),
        "pb": nc.declare_dram_parameter("pb", [1, C], BF16, isOutput=False),
        "rpbf": nc.declare_dram_parameter("rpbf", [HEADS, 66, 128], BF16, isOutput=False),
        "maskr": nc.declare_dram_parameter("maskr", [66, 512], BF16, isOutput=False),
        "ident": nc.declare_dram_parameter("ident", [128, 128], BF16, isOutput=False),
        "ident8": nc.declare_dram_parameter("ident8", [128, 128], FP8, isOutput=False),
        "ones": nc.declare_dram_parameter("ones", [1, 128], BF16, isOutput=False),
        "out": nc.declare_dram_parameter("out", [T, C], F32, isOutput=True),
        "_sw1": sw1,
    }
    with tile.TileContext(nc) as tc:
        _emit(tc, io, T, stage, reps)
    nc.compile()
    return nc


def host_prep(inputs):
    """Shared (replicated) host-side tensors from the raw inputs."""
    wq = inputs["wq"]
    wkv = inputs["wkv"]
    g = inputs["norm1_g"].astype(np.float32)
    b = inputs["norm1_b"].astype(np.float32)
    # fold layernorm affine into the projection weights/biases (exact when
    # g == 1 and b == 0, which is what setup_inputs produces)
    wq_f = (g[:, None] * wq).astype(np.float32)
    wkv_f = (g[:, None] * wkv).astype(np.float32)
    bq_f = (inputs["bq"] + b @ wq).astype(np.float32)
    bkv_f = (inputs["bkv"] + b @ wkv).astype(np.float32)

    wk_f, wv_f = wkv_f[:, :C], wkv_f[:, C:]
    bk_f, bv_f = bkv_f[:C], bkv_f[C:]

    idx = _rel_pos_index(WIN)
    rpb = inputs["bias_table"][idx.reshape(-1)].reshape(N, N, HEADS)
    rpb = rpb.transpose(2, 0, 1).astype(np.float32)       # [h, q, k]

    rpbf = np.zeros((HEADS, 66, 128), np.float32)
    for h in range(HEADS):
        # rows 0..63: row i, col q -> rpb[h][q mod 64, i]
        rpbf[h, :64, :64] = rpb[h].T
        rpbf[h, :64, 64:] = rpb[h].T
    rpbf[:, 64, :64] = 1.0     # q in first window
    rpbf[:, 65, 64:] = 1.0     # q in second window
    rpbf *= SQK                # bias lands on the SQK-scaled logits
    maskr = np.zeros((66, 512), np.float32)
    eye = np.eye(64, dtype=np.float32)
    for j in range(4):
        maskr[:64, 128 * j:128 * j + 64] = eye
        maskr[:64, 128 * j + 64:128 * j + 128] = eye
        maskr[64, 128 * j + 64:128 * j + 128] = NEG   # q-win0 x k-win1
        maskr[65, 128 * j:128 * j + 64] = NEG         # q-win1 x k-win0

    wm = inputs["w_mix"].astype(np.float32)
    e = np.exp(wm - wm.max())
    wmix = e / e.sum()
    w0, w1 = float(wmix[0]), float(wmix[1])

    def pack8(w):
        return np.ascontiguousarray(
            (w * SW).reshape(2, 2, 128, C).transpose(0, 2, 1, 3)).astype(F8)

    shared = {
        "wq": pack8(wq_f), "wk": pack8(wk_f), "wv": pack8(wv_f),
        "wp": pack8(w0 * inputs["proj_w"].astype(np.float32)),
        "bq": (bq_f * SCALE * SQ).reshape(C, 1), "bk": (bk_f * SK).reshape(C, 1),
        "bv": (bv_f * SW).reshape(1, C).astype(BF),
        "pb": (inputs["proj_b"].astype(np.float32) * SW * SO).reshape(1, C).astype(BF),
        "rpbf": rpbf.astype(BF), "maskr": maskr.astype(BF),
        "ident": np.eye(128, dtype=np.float32).astype(BF),
        "ident8": np.eye(128, dtype=np.float32).astype(F8),
        "ones": np.ones((1, 128), np.float32).astype(BF),
    }
    return shared, float(np.sqrt(w1 / w0))


def window_partition(x):
    xw = x.reshape(B, H // WIN, WIN, W // WIN, WIN, C)
    xw = xw.transpose(0, 1, 3, 2, 4, 5).reshape(-1, N, C)
    return np.ascontiguousarray(xw.reshape(-1, C))


def window_reverse(flat):
    o = flat.reshape(B, H // WIN, W // WIN, WIN, WIN, C)
    o = o.transpose(0, 1, 3, 2, 4, 5).reshape(B, H * W, C)
    return np.ascontiguousarray(o)


_NC_CACHE = {}
LAST_RESULTS = None


def kernel(**inputs):
    from concourse.bass_utils import run_bass_kernel_spmd

    inputs = {k: np.asarray(v) for k, v in inputs.items()}
    shared, sw1 = host_prep(inputs)
    xw = window_partition(inputs["x"].astype(np.float32))   # [131072, 512]
    T = xw.shape[0] // NCORES

    key = (T, round(sw1, 6))
    if key not in _NC_CACHE:
        _NC_CACHE[key] = build_nc(T, sw1=sw1)
    nc = _NC_CACHE[key]

    in_maps = []
    for c in range(NCORES):
        m = dict(shared)
        m["x"] = np.ascontiguousarray(xw[c * T:(c + 1) * T])
        in_maps.append(m)
    res = run_bass_kernel_spmd(nc, in_maps, list(range(NCORES)))
    global LAST_RESULTS
    LAST_RESULTS = res
    out = np.concatenate([res.results[c]["out"] for c in range(NCORES)], axis=0)
    return window_reverse(out).astype(np.float32)
